# revision 2
# baseline (speedup 1.0000x reference)
"""Single-head attention on 8 TRN2 NeuronCores, batch-parallel (1 batch elem/core).

reference (per batch b):
  qp = q[b] @ w_q; kp = k[b] @ w_k; vp = v[b] @ w_v        # [S,F]@[F,DK] -> [S,DK]
  scores = qp @ kp.T / sqrt(DK)                            # [S,S]
  out[b] = softmax(scores, axis=-1) @ vp                   # [S,DK]

Shapes: B=8, S=2048, F=1024, DK=128. f32 in/out, bf16 compute, f32 accumulate.

v3 structure (per core) — engine-balanced:
  * q,k transposed on the PE (identity matmuls); v transposed with the DMA
    XBAR (dma_start_transpose, block-mapped 3D out probed on HW), freeing
    ~33K PE cycles and putting them on the otherwise-idle DMA engines.
  * v projected wide (wv stationary, xTv moving, 512-wide streams) into
    vpT [DK, S]; an XBAR pass relays it to vp1 [sk, tile, DK] for PV.
  * PV reformulated: vp tile stationary, expT moving 512-wide — 64 matmuls
    of (128 ldw + 512 stream) instead of 256 of (128 + 129). Output lands
    transposed (outT [DK, sq]) in psum, accumulated over all 16 k-tiles
    with start/stop flags (2 chunks live at a time = 2 psum banks/pass).
  * softmax denominator: DVE accumulates colsum[sk,sq] += expT tile (f32),
    then 16 tiny PE matmuls (colsum_chunk.T @ ones) -> denom on sq
    partitions; reciprocal on DVE; applied by ACT during the final
    per-tile back-transpose (PE) of outT.
"""
import numpy as np

B, S, F, DK = 8, 2048, 1024, 128
P = 128
N_CORES = 8
GT = 4                 # s-tiles per group
NG = S // (P * GT)     # 4 groups per input
NF = F // P            # 8 f-chunks
NT = S // P            # 16 s-tiles
W4 = GT * P            # 512
SOFTMAX_SCALE = 1.0 / float(np.sqrt(DK))

_COMPILED = {}


def _build():
    import concourse.bass as bass
    import concourse.mybir as mybir
    from concourse import bacc
    from concourse.tile import TileContext
    from concourse.masks import make_identity

    f32 = mybir.dt.float32
    bf16 = mybir.dt.bfloat16
    EXP = mybir.ActivationFunctionType.Exp
    CPY = mybir.ActivationFunctionType.Copy
    ADD = mybir.AluOpType.add

    nc = bacc.Bacc("TRN2", target_bir_lowering=False, debug=False,
                   num_devices=N_CORES)
    q_ext = nc.dram_tensor("q", [S, F], f32, kind="ExternalInput").ap()
    k_ext = nc.dram_tensor("k", [S, F], f32, kind="ExternalInput").ap()
    v_ext = nc.dram_tensor("v", [S, F], f32, kind="ExternalInput").ap()
    wq_ext = nc.dram_tensor("w_q", [F, DK], f32, kind="ExternalInput").ap()
    wk_ext = nc.dram_tensor("w_k", [F, DK], f32, kind="ExternalInput").ap()
    wv_ext = nc.dram_tensor("w_v", [F, DK], f32, kind="ExternalInput").ap()
    out_ext = nc.dram_tensor("out", [S, DK], f32, kind="ExternalOutput").ap()

    q_view = q_ext.rearrange("(n p) f -> p n f", p=P)
    k_view = k_ext.rearrange("(n p) f -> p n f", p=P)
    v_view = v_ext.rearrange("(n p) f -> p n f", p=P)
    out_view = out_ext.rearrange("(n p) d -> p n d", p=P)

    with TileContext(nc) as tc:
        with (
            tc.tile_pool(name="const", bufs=1) as const,
            tc.tile_pool(name="persist", bufs=1) as persist,
            tc.tile_pool(name="stage", bufs=4) as stage,
            tc.tile_pool(name="xtp", bufs=2) as xtp,
            tc.tile_pool(name="xtv", bufs=2) as xtv,
            tc.tile_pool(name="outp", bufs=4) as outp,
            tc.tile_pool(name="tp_ps", bufs=2, space="PSUM") as tp_ps,
            tc.tile_pool(name="pj_ps", bufs=2, space="PSUM") as pj_ps,
            tc.tile_pool(name="sc_ps", bufs=2, space="PSUM") as sc_ps,
            tc.tile_pool(name="ac_ps", bufs=1, space="PSUM") as ac_ps,
        ):
            ident = const.tile([P, P], bf16)
            make_identity(nc, ident)
            ones_col = const.tile([P, 1], f32)
            nc.gpsimd.memset(ones_col[:, :], 1.0)

            def load_group(view, g, nm):
                x_nat = stage.tile([P, GT, F], bf16, tag="stage", name=nm)
                h = GT // 2
                for hh in range(2):
                    nc.gpsimd.dma_start(
                        out=x_nat[:, hh * h:(hh + 1) * h, :],
                        in_=view[:, GT * g + hh * h:GT * g + (hh + 1) * h, :])
                return x_nat

            # first q group load goes ahead of the (strided, slow) weight DMAs
            q_nat0 = load_group(q_view, 0, "q_nat")

            wq_sb = const.tile([P, NF, DK], bf16)
            nc.gpsimd.dma_start(out=wq_sb[:, :, :],
                                in_=wq_ext.rearrange("(c p) d -> p c d", p=P))
            wk_sb = const.tile([P, NF, DK], bf16)
            nc.gpsimd.dma_start(out=wk_sb[:, :, :],
                                in_=wk_ext.rearrange("(c p) d -> p c d", p=P))
            wv_sb = const.tile([P, NF, DK], bf16)
            nc.gpsimd.dma_start(out=wv_sb[:, :, :],
                                in_=wv_ext.rearrange("(c p) d -> p c d", p=P))

            qpT = persist.tile([P, S], bf16)           # [DK, sq]
            kpT = persist.tile([P, S], bf16)           # [DK, sk]
            vpT_sb = persist.tile([P, S], bf16)        # [DK, sk]
            vp1 = persist.tile([P, NT, P], bf16)       # [sk, tile, DK]
            expT_all = persist.tile([P, NT, S], bf16)  # [sk, sk-tile, sq]
            colsum = persist.tile([P, S], f32)         # [sk, sq] partial denom
            rinv_all = persist.tile([P, NT], f32)      # [sq, tile]
            outT_sb = persist.tile([P, 4, W4], bf16)   # [DK, sq-chunk, 512]

            def transpose_group(x_nat, nm):
                # [P, GT, F] bf16 (s on parts) -> [P, NF, GT*P] (f on parts)
                xT = xtp.tile([P, NF, W4], bf16, tag="xT", name=nm)
                for cc in range(NF // 2):
                    tp = tp_ps.tile([P, 2, W4], bf16, tag="tp", name="tp")
                    for ci in range(2):
                        c = 2 * cc + ci
                        for t in range(GT):
                            nc.tensor.transpose(
                                tp[:, ci, t * P:(t + 1) * P],
                                x_nat[:, t, c * P:(c + 1) * P],
                                ident[:, :])
                    nc.vector.tensor_copy(xT[:, 2 * cc:2 * cc + 2, :], tp[:, :, :])
                return xT

            def proj_qk(xT, w_sb, dstT, g):
                pj = pj_ps.tile([P, W4], f32, tag="pj", name="pj")
                for c in range(NF):
                    nc.tensor.matmul(pj[:, :], w_sb[:, c, :], xT[:, c, :],
                                     start=(c == 0), stop=(c == NF - 1))
                nc.scalar.copy(dstT[:, W4 * g:W4 * (g + 1)], pj[:, :])

            def scores_exp(t):
                for c in range(4):
                    sc = sc_ps.tile([P, W4], f32, tag="sc", name="sc")
                    nc.tensor.matmul(sc[:, :],
                                     kpT[:, t * P:(t + 1) * P],
                                     qpT[:, W4 * c:W4 * (c + 1)],
                                     start=True, stop=True)
                    nc.scalar.activation(
                        expT_all[:, t, W4 * c:W4 * (c + 1)],
                        sc[:, :], EXP, scale=SOFTMAX_SCALE)

            # outT accumulators: [DK, 512] per sq-chunk, 2 chunks per pass
            def pv_pass(chunks):
                accs = [
                    ac_ps.tile([P, W4], f32, tag=f"out{i}", name=f"out{i}")
                    for i in range(2)
                ]
                def step(t):
                    for i, c in enumerate(chunks):
                        nc.tensor.matmul(
                            accs[i][:, :],
                            vp1[:, t, :],
                            expT_all[:, t, W4 * c:W4 * (c + 1)],
                            start=(t == 0), stop=(t == NT - 1),
                            skip_group_check=True)
                return accs, step

            def finish(j):
                tp = tp_ps.tile([P, 2, W4], bf16, tag="tp", name="ftp")
                nc.tensor.transpose(
                    tp[:, 0, 0:P],
                    outT_sb[:, j // GT, (j % GT) * P:(j % GT + 1) * P],
                    ident[:, :])
                out_t = outp.tile([P, DK], f32, tag="out", name="out_t")
                nc.scalar.activation(out_t[:, :], tp[:, 0, 0:P], CPY,
                                     scale=rinv_all[:, j:j + 1])
                nc.sync.dma_start(out=out_view[:, j, :], in_=out_t[:, :])

            # ---- Q phase ----
            for g in range(NG):
                x_nat = q_nat0 if g == 0 else load_group(q_view, g, "q_nat")
                xT = transpose_group(x_nat, "qT")
                proj_qk(xT, wq_sb, qpT, g)

            # ---- K/V streaming; PV pass 1 (sq chunks 0,1) ----
            accs1, pv1 = pv_pass((0, 1))
            for g in range(NG):
                xTk = transpose_group(load_group(k_view, g, "k_nat"), "kT")
                proj_qk(xTk, wk_sb, kpT, g)

                v_nat = load_group(v_view, g, "v_nat")
                xTv = xtv.tile([P, NF, W4], bf16, tag="xTv", name="xTv")
                for tl in range(GT):
                    nc.sync.dma_start_transpose(
                        out=xTv[:, :, tl * P:(tl + 1) * P],
                        in_=v_nat[:, tl, :])
                vps = pj_ps.tile([P, W4], f32, tag="pj", name="vps")
                for c in range(NF):
                    nc.tensor.matmul(vps[:, :], wv_sb[:, c, :], xTv[:, c, :],
                                     start=(c == 0), stop=(c == NF - 1))
                nc.scalar.copy(vpT_sb[:, W4 * g:W4 * (g + 1)], vps[:, :])
                nc.sync.dma_start_transpose(
                    out=vp1[:, GT * g:GT * (g + 1), :],
                    in_=vpT_sb[:, W4 * g:W4 * (g + 1)])

                for t in range(GT * g, GT * (g + 1)):
                    scores_exp(t)
                    if t == 0:
                        nc.vector.tensor_copy(colsum[:, :], expT_all[:, 0, :])
                    else:
                        nc.vector.tensor_tensor(colsum[:, :], colsum[:, :],
                                                expT_all[:, t, :], ADD)
                    pv1(t)

            # ---- tail ----
            for i in range(2):
                nc.vector.tensor_copy(outT_sb[:, i, :], accs1[i][:, :])

            accs2, pv2 = pv_pass((2, 3))
            for t in range(NT):
                pv2(t)

            # denominator: 16 tiny matmuls colsum_chunk.T @ ones -> [sq, 1]
            for j in range(NT):
                dn = sc_ps.tile([P, W4], f32, tag="sc", name="dn")
                nc.tensor.matmul(dn[:, 0:1],
                                 colsum[:, j * P:(j + 1) * P],
                                 ones_col[:, :], start=True, stop=True)
                nc.vector.reciprocal(rinv_all[:, j:j + 1], dn[:, 0:1])

            for j in range(8):
                finish(j)
            for i in range(2):
                nc.vector.tensor_copy(outT_sb[:, 2 + i, :], accs2[i][:, :])
            for j in range(8, 16):
                finish(j)

    nc.compile()
    return nc


def get_nc():
    if "nc" not in _COMPILED:
        _COMPILED["nc"] = _build()
    return _COMPILED["nc"]


def kernel(q, k, v, w_q, w_k, w_v):
    from concourse.bass_utils import run_bass_kernel_spmd

    q = np.ascontiguousarray(np.asarray(q, dtype=np.float32))
    k = np.ascontiguousarray(np.asarray(k, dtype=np.float32))
    v = np.ascontiguousarray(np.asarray(v, dtype=np.float32))
    w_q = np.ascontiguousarray(np.asarray(w_q, dtype=np.float32))
    w_k = np.ascontiguousarray(np.asarray(w_k, dtype=np.float32))
    w_v = np.ascontiguousarray(np.asarray(w_v, dtype=np.float32))

    nc = get_nc()
    in_maps = [
        {"q": q[b], "k": k[b], "v": v[b], "w_q": w_q, "w_k": w_k, "w_v": w_v}
        for b in range(B)
    ]
    res = run_bass_kernel_spmd(nc, in_maps, core_ids=list(range(N_CORES)))
    out = np.stack([res.results[b]["out"] for b in range(B)], axis=0)
    return out.astype(np.float32)


# revision 3
# speedup vs baseline: 1.4556x; 1.4556x over previous
"""Single-head attention on 8 TRN2 NeuronCores, batch-parallel (1 batch elem/core).

reference (per batch b):
  qp = q[b] @ w_q; kp = k[b] @ w_k; vp = v[b] @ w_v        # [S,F]@[F,DK] -> [S,DK]
  scores = qp @ kp.T / sqrt(DK)                            # [S,S]
  out[b] = softmax(scores, axis=-1) @ vp                   # [S,DK]

Shapes: B=8, S=2048, F=1024, DK=128. f32 in/out, bf16 compute, f32 accumulate.

v4 structure (per core):
  * q,k,v streamed per group, PE-transposed (bf16, f32->bf16 cast in the
    SWDGE load). DMA measured at per-engine line rate (22.5 B/ns read), so
    the 25MB input read is a ~75us DMA floor; XBAR transposes were tried
    and starved the PE behind saturated DMA queues (v3: 188us) - keep all
    transposes on the PE, whose LDWEIGHTS run on a separate, overlapped
    queue (measured: back-to-back matmuls overlap to stream-rate).
  * PV reformulated vs the 135us baseline: vp tile stationary, expT moving
    512-wide - 64 matmuls of 512-col streams instead of 256 of 129. Output
    lands transposed (outT [DK, sq]) in psum, accumulated across all 16
    k-tiles via start/stop flags; 2 sq-chunks per pass, 2 passes.
  * softmax denominator: DVE accumulates colsum[sk,sq] += expT tile (f32);
    16 tiny PE matmuls (colsum_chunk.T @ ones) put denom on sq partitions;
    DVE reciprocal; ACT applies the scale during the final per-tile
    back-transpose of outT.
"""
import numpy as np

B, S, F, DK = 8, 2048, 1024, 128
P = 128
N_CORES = 8
GT = 4                 # s-tiles per group
NG = S // (P * GT)     # 4 groups per input
NF = F // P            # 8 f-chunks
NT = S // P            # 16 s-tiles
W4 = GT * P            # 512
SOFTMAX_SCALE = 1.0 / float(np.sqrt(DK))

_COMPILED = {}


def _build():
    import concourse.bass as bass
    import concourse.mybir as mybir
    from concourse import bacc
    from concourse.tile import TileContext
    from concourse.masks import make_identity

    f32 = mybir.dt.float32
    bf16 = mybir.dt.bfloat16
    EXP = mybir.ActivationFunctionType.Exp
    CPY = mybir.ActivationFunctionType.Copy
    ADD = mybir.AluOpType.add

    nc = bacc.Bacc("TRN2", target_bir_lowering=False, debug=False,
                   num_devices=N_CORES)
    q_ext = nc.dram_tensor("q", [S, F], f32, kind="ExternalInput").ap()
    k_ext = nc.dram_tensor("k", [S, F], f32, kind="ExternalInput").ap()
    v_ext = nc.dram_tensor("v", [S, F], f32, kind="ExternalInput").ap()
    wq_ext = nc.dram_tensor("w_q", [F, DK], f32, kind="ExternalInput").ap()
    wk_ext = nc.dram_tensor("w_k", [F, DK], f32, kind="ExternalInput").ap()
    wv_ext = nc.dram_tensor("w_v", [F, DK], f32, kind="ExternalInput").ap()
    out_ext = nc.dram_tensor("out", [S, DK], f32, kind="ExternalOutput").ap()

    q_view = q_ext.rearrange("(n p) f -> p n f", p=P)
    k_view = k_ext.rearrange("(n p) f -> p n f", p=P)
    v_view = v_ext.rearrange("(n p) f -> p n f", p=P)
    out_view = out_ext.rearrange("(n p) d -> p n d", p=P)

    with TileContext(nc) as tc:
        with (
            tc.tile_pool(name="const", bufs=1) as const,
            tc.tile_pool(name="persist", bufs=1) as persist,
            tc.tile_pool(name="stage", bufs=4) as stage,
            tc.tile_pool(name="xtp", bufs=2) as xtp,
            tc.tile_pool(name="outp", bufs=4) as outp,
            tc.tile_pool(name="tp_ps", bufs=2, space="PSUM") as tp_ps,
            tc.tile_pool(name="pj_ps", bufs=2, space="PSUM") as pj_ps,
            tc.tile_pool(name="sc_ps", bufs=2, space="PSUM") as sc_ps,
            tc.tile_pool(name="ac_ps", bufs=1, space="PSUM") as ac_ps,
        ):
            ident = const.tile([P, P], bf16)
            make_identity(nc, ident)
            ones_col = const.tile([P, 1], f32)
            nc.gpsimd.memset(ones_col[:, :], 1.0)

            def load_group(view, g, nm):
                x_nat = stage.tile([P, GT, F], bf16, tag="stage", name=nm)
                h = GT // 2
                for hh in range(2):
                    nc.gpsimd.dma_start(
                        out=x_nat[:, hh * h:(hh + 1) * h, :],
                        in_=view[:, GT * g + hh * h:GT * g + (hh + 1) * h, :])
                return x_nat

            # first q group load goes ahead of the (strided, slow) weight DMAs
            q_nat0 = load_group(q_view, 0, "q_nat")

            wq_sb = const.tile([P, NF, DK], bf16)
            nc.gpsimd.dma_start(out=wq_sb[:, :, :],
                                in_=wq_ext.rearrange("(c p) d -> p c d", p=P))
            wk_sb = const.tile([P, NF, DK], bf16)
            nc.gpsimd.dma_start(out=wk_sb[:, :, :],
                                in_=wk_ext.rearrange("(c p) d -> p c d", p=P))
            wv_sb = const.tile([P, NF, DK], bf16)
            nc.gpsimd.dma_start(out=wv_sb[:, :, :],
                                in_=wv_ext.rearrange("(c p) d -> p c d", p=P))

            qpT = persist.tile([P, S], bf16)           # [DK, sq]
            kpT = persist.tile([P, S], bf16)           # [DK, sk]
            vp1 = persist.tile([P, NT, DK], bf16)      # [sk, tile, DK]
            expT_all = persist.tile([P, NT, S], bf16)  # [sk, sk-tile, sq]
            colsum = persist.tile([P, S], f32)         # [sk, sq] partial denom
            rinv_all = persist.tile([P, NT], f32)      # [sq, tile]
            outT_sb = persist.tile([P, 4, W4], bf16)   # [DK, sq-chunk, 512]

            def transpose_group(x_nat, nm):
                # [P, GT, F] bf16 (s on parts) -> [P, NF, GT*P] (f on parts)
                xT = xtp.tile([P, NF, W4], bf16, tag="xT", name=nm)
                for cc in range(NF // 2):
                    tp = tp_ps.tile([P, 2, W4], bf16, tag="tp", name="tp")
                    for ci in range(2):
                        c = 2 * cc + ci
                        for t in range(GT):
                            nc.tensor.transpose(
                                tp[:, ci, t * P:(t + 1) * P],
                                x_nat[:, t, c * P:(c + 1) * P],
                                ident[:, :])
                    nc.vector.tensor_copy(xT[:, 2 * cc:2 * cc + 2, :], tp[:, :, :])
                return xT

            def proj_qk(xT, w_sb, dstT, g):
                pj = pj_ps.tile([P, W4], f32, tag="pj", name="pj")
                for c in range(NF):
                    nc.tensor.matmul(pj[:, :], w_sb[:, c, :], xT[:, c, :],
                                     start=(c == 0), stop=(c == NF - 1))
                nc.scalar.copy(dstT[:, W4 * g:W4 * (g + 1)], pj[:, :])

            def proj_v(xT, g):
                for tl in range(GT):
                    vps = pj_ps.tile([P, W4], f32, tag="pj", name="vps")
                    for c in range(NF):
                        nc.tensor.matmul(vps[:, 0:DK],
                                         xT[:, c, tl * P:(tl + 1) * P],
                                         wv_sb[:, c, :],
                                         start=(c == 0), stop=(c == NF - 1))
                    nc.vector.tensor_copy(vp1[:, GT * g + tl, :], vps[:, 0:DK])

            def scores_exp(t):
                for c in range(4):
                    sc = sc_ps.tile([P, W4], f32, tag="sc", name="sc")
                    nc.tensor.matmul(sc[:, :],
                                     kpT[:, t * P:(t + 1) * P],
                                     qpT[:, W4 * c:W4 * (c + 1)],
                                     start=True, stop=True)
                    nc.scalar.activation(
                        expT_all[:, t, W4 * c:W4 * (c + 1)],
                        sc[:, :], EXP, scale=SOFTMAX_SCALE)

            # outT accumulators: [DK, 512] per sq-chunk, 2 chunks per pass
            def pv_pass(chunks):
                accs = [
                    ac_ps.tile([P, W4], f32, tag=f"out{i}", name=f"out{i}")
                    for i in range(2)
                ]
                def step(t):
                    for i, c in enumerate(chunks):
                        nc.tensor.matmul(
                            accs[i][:, :],
                            vp1[:, t, :],
                            expT_all[:, t, W4 * c:W4 * (c + 1)],
                            start=(t == 0), stop=(t == NT - 1),
                            skip_group_check=True)
                return accs, step

            def finish(j):
                tp = tp_ps.tile([P, 2, W4], bf16, tag="tp", name="ftp")
                nc.tensor.transpose(
                    tp[:, 0, 0:P],
                    outT_sb[:, j // GT, (j % GT) * P:(j % GT + 1) * P],
                    ident[:, :])
                out_t = outp.tile([P, DK], f32, tag="out", name="out_t")
                nc.scalar.activation(out_t[:, :], tp[:, 0, 0:P], CPY,
                                     scale=rinv_all[:, j:j + 1])
                nc.sync.dma_start(out=out_view[:, j, :], in_=out_t[:, :])

            # ---- Q phase ----
            for g in range(NG):
                x_nat = q_nat0 if g == 0 else load_group(q_view, g, "q_nat")
                xT = transpose_group(x_nat, "qT")
                proj_qk(xT, wq_sb, qpT, g)

            # ---- K/V streaming; PV pass 1 (sq chunks 0,1) ----
            accs1, pv1 = pv_pass((0, 1))
            for g in range(NG):
                xTk = transpose_group(load_group(k_view, g, "k_nat"), "kT")
                proj_qk(xTk, wk_sb, kpT, g)
                xTv = transpose_group(load_group(v_view, g, "v_nat"), "vT")
                proj_v(xTv, g)
                for t in range(GT * g, GT * (g + 1)):
                    scores_exp(t)
                    if t == 0:
                        nc.vector.tensor_copy(colsum[:, :], expT_all[:, 0, :])
                    else:
                        nc.vector.tensor_tensor(colsum[:, :], colsum[:, :],
                                                expT_all[:, t, :], ADD)
                    pv1(t)

            # ---- tail ----
            for i in range(2):
                nc.vector.tensor_copy(outT_sb[:, i, :], accs1[i][:, :])

            # denominator: 16 tiny matmuls colsum_chunk.T @ ones -> [sq, 1]
            for j in range(NT):
                dn = sc_ps.tile([P, W4], f32, tag="sc", name="dn")
                nc.tensor.matmul(dn[:, 0:1],
                                 colsum[:, j * P:(j + 1) * P],
                                 ones_col[:, :], start=True, stop=True)
                nc.vector.reciprocal(rinv_all[:, j:j + 1], dn[:, 0:1])

            accs2, pv2 = pv_pass((2, 3))
            for t in range(NT):
                pv2(t)
                if t == 7:
                    # drain pass-1 outputs while pass-2 PV streams
                    for j in range(8):
                        finish(j)
            for i in range(2):
                nc.vector.tensor_copy(outT_sb[:, 2 + i, :], accs2[i][:, :])
            for j in range(8, 16):
                finish(j)

    nc.compile()
    return nc


def get_nc():
    if "nc" not in _COMPILED:
        _COMPILED["nc"] = _build()
    return _COMPILED["nc"]


def kernel(q, k, v, w_q, w_k, w_v):
    from concourse.bass_utils import run_bass_kernel_spmd

    q = np.ascontiguousarray(np.asarray(q, dtype=np.float32))
    k = np.ascontiguousarray(np.asarray(k, dtype=np.float32))
    v = np.ascontiguousarray(np.asarray(v, dtype=np.float32))
    w_q = np.ascontiguousarray(np.asarray(w_q, dtype=np.float32))
    w_k = np.ascontiguousarray(np.asarray(w_k, dtype=np.float32))
    w_v = np.ascontiguousarray(np.asarray(w_v, dtype=np.float32))

    nc = get_nc()
    in_maps = [
        {"q": q[b], "k": k[b], "v": v[b], "w_q": w_q, "w_k": w_k, "w_v": w_v}
        for b in range(B)
    ]
    res = run_bass_kernel_spmd(nc, in_maps, core_ids=list(range(N_CORES)))
    out = np.stack([res.results[b]["out"] for b in range(B)], axis=0)
    return out.astype(np.float32)


# revision 6
# speedup vs baseline: 1.4558x; 1.0001x over previous
"""Single-head attention on 8 TRN2 NeuronCores, batch-parallel (1 batch elem/core).

reference (per batch b):
  qp = q[b] @ w_q; kp = k[b] @ w_k; vp = v[b] @ w_v        # [S,F]@[F,DK] -> [S,DK]
  scores = qp @ kp.T / sqrt(DK)                            # [S,S]
  out[b] = softmax(scores, axis=-1) @ vp                   # [S,DK]

Shapes: B=8, S=2048, F=1024, DK=128. f32 in/out, bf16 compute, f32 accumulate.

v4 structure (per core):
  * q,k,v streamed per group, PE-transposed (bf16, f32->bf16 cast in the
    SWDGE load). DMA measured at per-engine line rate (22.5 B/ns read), so
    the 25MB input read is a ~75us DMA floor; XBAR transposes were tried
    and starved the PE behind saturated DMA queues (v3: 188us) - keep all
    transposes on the PE, whose LDWEIGHTS run on a separate, overlapped
    queue (measured: back-to-back matmuls overlap to stream-rate).
  * PV reformulated vs the 135us baseline: vp tile stationary, expT moving
    512-wide - 64 matmuls of 512-col streams instead of 256 of 129. Output
    lands transposed (outT [DK, sq]) in psum, accumulated across all 16
    k-tiles via start/stop flags; 2 sq-chunks per pass, 2 passes.
  * softmax denominator: DVE accumulates colsum[sk,sq] += expT tile (f32);
    16 tiny PE matmuls (colsum_chunk.T @ ones) put denom on sq partitions;
    DVE reciprocal; ACT applies the scale during the final per-tile
    back-transpose of outT.
"""
import numpy as np

B, S, F, DK = 8, 2048, 1024, 128
P = 128
N_CORES = 8
GT = 4                 # s-tiles per group
NG = S // (P * GT)     # 4 groups per input
NF = F // P            # 8 f-chunks
NT = S // P            # 16 s-tiles
W4 = GT * P            # 512
SOFTMAX_SCALE = 1.0 / float(np.sqrt(DK))

_COMPILED = {}


def _build():
    import concourse.bass as bass
    import concourse.mybir as mybir
    from concourse import bacc
    from concourse.tile import TileContext
    from concourse.masks import make_identity

    f32 = mybir.dt.float32
    bf16 = mybir.dt.bfloat16
    EXP = mybir.ActivationFunctionType.Exp
    CPY = mybir.ActivationFunctionType.Copy
    ADD = mybir.AluOpType.add

    nc = bacc.Bacc("TRN2", target_bir_lowering=False, debug=False,
                   num_devices=N_CORES)
    q_ext = nc.dram_tensor("q", [S, F], f32, kind="ExternalInput").ap()
    k_ext = nc.dram_tensor("k", [S, F], f32, kind="ExternalInput").ap()
    v_ext = nc.dram_tensor("v", [S, F], f32, kind="ExternalInput").ap()
    wq_ext = nc.dram_tensor("w_q", [F, DK], f32, kind="ExternalInput").ap()
    wk_ext = nc.dram_tensor("w_k", [F, DK], f32, kind="ExternalInput").ap()
    wv_ext = nc.dram_tensor("w_v", [F, DK], f32, kind="ExternalInput").ap()
    out_ext = nc.dram_tensor("out", [S, DK], f32, kind="ExternalOutput").ap()

    q_view = q_ext.rearrange("(n p) f -> p n f", p=P)
    k_view = k_ext.rearrange("(n p) f -> p n f", p=P)
    v_view = v_ext.rearrange("(n p) f -> p n f", p=P)
    out_view = out_ext.rearrange("(n p) d -> p n d", p=P)

    with TileContext(nc) as tc:
        with (
            tc.tile_pool(name="const", bufs=1) as const,
            tc.tile_pool(name="persist", bufs=1) as persist,
            tc.tile_pool(name="stage", bufs=4) as stage,
            tc.tile_pool(name="xtp", bufs=2) as xtp,
            tc.tile_pool(name="outp", bufs=4) as outp,
            tc.tile_pool(name="tp_ps", bufs=2, space="PSUM") as tp_ps,
            tc.tile_pool(name="pj_ps", bufs=2, space="PSUM") as pj_ps,
            tc.tile_pool(name="sc_ps", bufs=2, space="PSUM") as sc_ps,
            tc.tile_pool(name="ac_ps", bufs=1, space="PSUM") as ac_ps,
        ):
            def load_group(view, g, nm):
                x_nat = stage.tile([P, GT, F], bf16, tag="stage", name=nm)
                h = GT // 2
                for hh in range(2):
                    nc.gpsimd.dma_start(
                        out=x_nat[:, hh * h:(hh + 1) * h, :],
                        in_=view[:, GT * g + hh * h:GT * g + (hh + 1) * h, :])
                return x_nat

            def load_weight(ext, nm):
                w_sb = const.tile([P, NF, DK], bf16, tag=nm, name=nm)
                nc.gpsimd.dma_start(out=w_sb[:, :, :],
                                    in_=ext.rearrange("(c p) d -> p c d", p=P))
                return w_sb

            # first q group load goes ahead of everything on the DMA queue
            q_nat0 = load_group(q_view, 0, "q_nat")
            wq_sb = load_weight(wq_ext, "wq")

            ident = const.tile([P, P], bf16)
            make_identity(nc, ident)
            ones_col = const.tile([P, 1], f32)
            nc.gpsimd.memset(ones_col[:, :], 1.0)

            qpT = persist.tile([P, S], bf16)           # [DK, sq]
            kpT = persist.tile([P, S], bf16)           # [DK, sk]
            vp1 = persist.tile([P, NT, DK], bf16)      # [sk, tile, DK]
            expT_all = persist.tile([P, NT, S], bf16)  # [sk, sk-tile, sq]
            colsum = persist.tile([P, S], f32)         # [sk, sq] partial denom
            rinv_all = persist.tile([P, NT], f32)      # [sq, tile]
            outT_sb = persist.tile([P, 4, W4], bf16)   # [DK, sq-chunk, 512]

            def transpose_group(x_nat, nm):
                # [P, GT, F] bf16 (s on parts) -> [P, NF, GT*P] (f on parts)
                xT = xtp.tile([P, NF, W4], bf16, tag="xT", name=nm)
                for cc in range(NF // 2):
                    tp = tp_ps.tile([P, 2, W4], bf16, tag="tp", name="tp")
                    for ci in range(2):
                        c = 2 * cc + ci
                        for t in range(GT):
                            nc.tensor.transpose(
                                tp[:, ci, t * P:(t + 1) * P],
                                x_nat[:, t, c * P:(c + 1) * P],
                                ident[:, :])
                    nc.vector.tensor_copy(xT[:, 2 * cc:2 * cc + 2, :], tp[:, :, :])
                return xT

            def proj_qk(xT, w_sb, dstT, g):
                pj = pj_ps.tile([P, W4], f32, tag="pj", name="pj")
                for c in range(NF):
                    nc.tensor.matmul(pj[:, :], w_sb[:, c, :], xT[:, c, :],
                                     start=(c == 0), stop=(c == NF - 1))
                nc.scalar.copy(dstT[:, W4 * g:W4 * (g + 1)], pj[:, :])

            def proj_v(xT, g):
                for tl in range(GT):
                    vps = pj_ps.tile([P, W4], f32, tag="pj", name="vps")
                    for c in range(NF):
                        nc.tensor.matmul(vps[:, 0:DK],
                                         xT[:, c, tl * P:(tl + 1) * P],
                                         wv_sb[:, c, :],
                                         start=(c == 0), stop=(c == NF - 1))
                    nc.vector.tensor_copy(vp1[:, GT * g + tl, :], vps[:, 0:DK])

            def scores_exp(t):
                for c in range(4):
                    sc = sc_ps.tile([P, W4], f32, tag="sc", name="sc")
                    nc.tensor.matmul(sc[:, :],
                                     kpT[:, t * P:(t + 1) * P],
                                     qpT[:, W4 * c:W4 * (c + 1)],
                                     start=True, stop=True)
                    nc.scalar.activation(
                        expT_all[:, t, W4 * c:W4 * (c + 1)],
                        sc[:, :], EXP, scale=SOFTMAX_SCALE)

            # outT accumulators: [DK, 512] per sq-chunk, 2 chunks per pass
            def pv_pass(chunks):
                accs = [
                    ac_ps.tile([P, W4], f32, tag=f"out{i}", name=f"out{i}")
                    for i in range(2)
                ]
                def step(t):
                    for i, c in enumerate(chunks):
                        nc.tensor.matmul(
                            accs[i][:, :],
                            vp1[:, t, :],
                            expT_all[:, t, W4 * c:W4 * (c + 1)],
                            start=(t == 0), stop=(t == NT - 1),
                            skip_group_check=True)
                return accs, step

            def finish(j):
                tp = tp_ps.tile([P, 2, W4], bf16, tag="tp", name="ftp")
                nc.tensor.transpose(
                    tp[:, 0, 0:P],
                    outT_sb[:, j // GT, (j % GT) * P:(j % GT + 1) * P],
                    ident[:, :])
                out_t = outp.tile([P, DK], f32, tag="out", name="out_t")
                nc.scalar.activation(out_t[:, :], tp[:, 0, 0:P], CPY,
                                     scale=rinv_all[:, j:j + 1])
                nc.sync.dma_start(out=out_view[:, j, :], in_=out_t[:, :])

            # ---- Q phase ----
            for g in range(NG):
                x_nat = q_nat0 if g == 0 else load_group(q_view, g, "q_nat")
                xT = transpose_group(x_nat, "qT")
                proj_qk(xT, wq_sb, qpT, g)

            # ---- K/V streaming; PV pass 1 (sq chunks 0,1) ----
            accs1, pv1 = pv_pass((0, 1))
            wk_sb = wv_sb = None
            for g in range(NG):
                k_nat = load_group(k_view, g, "k_nat")
                if g == 0:
                    # weight DMAs deferred behind the first k/v tile loads
                    wk_sb = load_weight(wk_ext, "wk")
                xTk = transpose_group(k_nat, "kT")
                proj_qk(xTk, wk_sb, kpT, g)
                v_nat = load_group(v_view, g, "v_nat")
                if g == 0:
                    wv_sb = load_weight(wv_ext, "wv")
                xTv = transpose_group(v_nat, "vT")
                proj_v(xTv, g)
                for t in range(GT * g, GT * (g + 1)):
                    scores_exp(t)
                    if t == 0:
                        nc.vector.tensor_copy(colsum[:, :], expT_all[:, 0, :])
                    else:
                        nc.vector.tensor_tensor(colsum[:, :], colsum[:, :],
                                                expT_all[:, t, :], ADD)
                    pv1(t)

            # ---- tail ----
            for i in range(2):
                nc.vector.tensor_copy(outT_sb[:, i, :], accs1[i][:, :])

            # denominator: 16 tiny matmuls colsum_chunk.T @ ones -> [sq, 1]
            for j in range(NT):
                dn = sc_ps.tile([P, W4], f32, tag="sc", name="dn")
                nc.tensor.matmul(dn[:, 0:1],
                                 colsum[:, j * P:(j + 1) * P],
                                 ones_col[:, :], start=True, stop=True)
                nc.vector.reciprocal(rinv_all[:, j:j + 1], dn[:, 0:1])

            # pass 2 split into two single-chunk sub-passes so the final
            # finishes interleave with PV instead of trailing the kernel
            def pv_single(chunk, tag):
                acc = ac_ps.tile([P, W4], f32, tag=tag, name=tag)
                for t in range(NT):
                    nc.tensor.matmul(
                        acc[:, :], vp1[:, t, :],
                        expT_all[:, t, W4 * chunk:W4 * (chunk + 1)],
                        start=(t == 0), stop=(t == NT - 1),
                        skip_group_check=True)
                return acc

            acc2 = pv_single(2, "out0")
            for j in range(4):
                finish(j)
            nc.vector.tensor_copy(outT_sb[:, 2, :], acc2[:, :])
            acc3 = pv_single(3, "out1")
            for j in range(4, 12):
                finish(j)
            nc.vector.tensor_copy(outT_sb[:, 3, :], acc3[:, :])
            for j in range(12, 16):
                finish(j)

    nc.compile()
    return nc


def get_nc():
    if "nc" not in _COMPILED:
        _COMPILED["nc"] = _build()
    return _COMPILED["nc"]


def kernel(q, k, v, w_q, w_k, w_v):
    from concourse.bass_utils import run_bass_kernel_spmd

    q = np.ascontiguousarray(np.asarray(q, dtype=np.float32))
    k = np.ascontiguousarray(np.asarray(k, dtype=np.float32))
    v = np.ascontiguousarray(np.asarray(v, dtype=np.float32))
    w_q = np.ascontiguousarray(np.asarray(w_q, dtype=np.float32))
    w_k = np.ascontiguousarray(np.asarray(w_k, dtype=np.float32))
    w_v = np.ascontiguousarray(np.asarray(w_v, dtype=np.float32))

    nc = get_nc()
    in_maps = [
        {"q": q[b], "k": k[b], "v": v[b], "w_q": w_q, "w_k": w_k, "w_v": w_v}
        for b in range(B)
    ]
    res = run_bass_kernel_spmd(nc, in_maps, core_ids=list(range(N_CORES)))
    out = np.stack([res.results[b]["out"] for b in range(B)], axis=0)
    return out.astype(np.float32)
